# revision 1
# baseline (speedup 1.0000x reference)
"""Trainium2 Bass kernel for nn_ContrastiveLoss (NT-Xent style contrastive loss).

Strategy (8 NeuronCores, SPMD):
  - Host sorts samples by label (the scalar loss is permutation invariant),
    row-normalizes, and builds X^T [D=128, N=8192] in bf16.
  - Rows are sharded across 8 cores (1024 rows each, 8 blocks of 128).
  - Each core computes its [1024, 8192] similarity block against the full
    X^T (the "all-gathered" copy arrives as a per-core input), reduces
    exp-row-sums on-chip, and evaluates the positive-pair terms only on a
    narrow label-band window (sorted labels make positives contiguous).
  - Per-row partial losses return to the host, which sums them and divides
    by the exact positive-pair count (from the label histogram).

Math: with e_ij = exp(sim_ij/T), S_i = sum_j e_ij (incl diag),
P_i = sum_{j in label-range(i)} e_ij (incl diag), unsim_i = S_i - P_i,
u_i = log(unsim_i), the reference loss row-sum equals
  npos_i*u_i + sum_{range} softplus(sim_ij/T - u_i) - softplus(1/T - u_i)
             - (sum_{range} sim_ij/T - 1/T)
where npos_i = (label count of i) - 1. The diagonal contributions cancel
exactly in unsim and are removed via the constant sim_ii = 1 (rows are
normalized; the fp difference is ~1e-9 relative on the final scalar).
"""

import numpy as np

T = 0.2
INV_T = 1.0 / T  # 5.0
EPS = 1e-5
N, D, NCLASS = 8192, 128, 128
NCORES = 8
ROWS_PER_CORE = N // NCORES          # 1024
BLOCKS = ROWS_PER_CORE // 128        # 8 blocks of 128 rows per core
CHUNK = 2048                         # ACT chunk (4 PSUM banks)
NCHUNKS = N // CHUNK                 # 4 per block
MM = 512                             # matmul free-dim per PSUM bank

_CACHE = {}


def _build_nc(W, debug=False):
    """Build the SPMD Bass/Tile program. W = band window width (mult of 512)."""
    import concourse.bass as bass
    import concourse.bacc as bacc
    import concourse.mybir as mybir
    import concourse.tile as tile

    dt = mybir.dt
    AF = mybir.ActivationFunctionType
    ALU = mybir.AluOpType
    X = mybir.AxisListType.X

    nc = bacc.Bacc("TRN2", target_bir_lowering=False, debug=debug)

    xt_d = nc.dram_tensor("xt", [128, N], dt.bfloat16, kind="ExternalInput")
    xtown_d = nc.dram_tensor("xtown", [128, ROWS_PER_CORE], dt.bfloat16,
                             kind="ExternalInput")
    xtband_d = nc.dram_tensor("xtband", [128, BLOCKS * W], dt.bfloat16,
                              kind="ExternalInput")
    gsr_d = nc.dram_tensor("gsr", [128, BLOCKS], dt.float32, kind="ExternalInput")
    ger_d = nc.dram_tensor("ger", [128, BLOCKS], dt.float32, kind="ExternalInput")
    npos_d = nc.dram_tensor("npos", [128, BLOCKS], dt.float32, kind="ExternalInput")
    out_d = nc.dram_tensor("out", [128, BLOCKS], dt.float32, kind="ExternalOutput")

    nwc = W // MM  # band matmul sub-chunks

    with tile.TileContext(nc) as tc:
        with (
            tc.tile_pool(name="const", bufs=1) as const,
            tc.tile_pool(name="band", bufs=1) as band,
            tc.tile_pool(name="etmp", bufs=3) as etmp_pool,
            tc.tile_pool(name="sp", bufs=2) as sp_pool,
            tc.tile_pool(name="small", bufs=1) as small,
            tc.tile_pool(name="psum", bufs=2, space="PSUM") as psum,
        ):
            # ---- persistent loads ----
            xt = const.tile([128, N], dt.bfloat16)
            for k in range(N // CHUNK):
                nc.sync.dma_start(xt[:, k * CHUNK:(k + 1) * CHUNK],
                                  xt_d[:, k * CHUNK:(k + 1) * CHUNK])
            xtown = const.tile([128, ROWS_PER_CORE], dt.bfloat16)
            nc.sync.dma_start(xtown[:], xtown_d[:])
            xtband = const.tile([128, BLOCKS * W], dt.bfloat16)
            nc.sync.dma_start(xtband[:], xtband_d[:])
            gsr = const.tile([128, BLOCKS], dt.float32)
            nc.sync.dma_start(gsr[:], gsr_d[:])
            ger = const.tile([128, BLOCKS], dt.float32)
            nc.sync.dma_start(ger[:], ger_d[:])
            npos = const.tile([128, BLOCKS], dt.float32)
            nc.sync.dma_start(npos[:], npos_d[:])

            iota_i = const.tile([128, W], dt.int32)
            nc.gpsimd.iota(iota_i[:], pattern=[[1, W]], base=0, channel_multiplier=0)
            iota_f = const.tile([128, W], dt.float32)
            nc.vector.tensor_copy(iota_f[:], iota_i[:])

            acc = const.tile([128, BLOCKS], dt.float32)

            # per-block persistent tiles
            s_band = [band.tile([128, W], dt.float32, name=f"sb{b}") for b in range(BLOCKS)]
            e_band = [band.tile([128, W], dt.float32, name=f"eb{b}") for b in range(BLOCKS)]
            mask = [band.tile([128, W], dt.float32, name=f"mk{b}") for b in range(BLOCKS)]
            S = [small.tile([128, 1], dt.float32, name=f"S{b}") for b in range(BLOCKS)]
            P = [small.tile([128, 1], dt.float32, name=f"P{b}") for b in range(BLOCKS)]
            u = [small.tile([128, 1], dt.float32, name=f"u{b}") for b in range(BLOCKS)]
            runsim = [small.tile([128, 1], dt.float32, name=f"ru{b}") for b in range(BLOCKS)]
            spd = [small.tile([128, 1], dt.float32, name=f"sd{b}") for b in range(BLOCKS)]
            sparts = [small.tile([128, NCHUNKS], dt.float32, name=f"sp{b}")
                      for b in range(BLOCKS)]

            # ---- Phase A: dense exp row-sums (Exp table) + band sims ----
            for b in range(BLOCKS):
                lhsT = xtown[:, b * 128:(b + 1) * 128]
                for kc in range(NCHUNKS):
                    ps = psum.tile([128, CHUNK], dt.float32, tag="ps")
                    for j in range(CHUNK // MM):
                        c0 = kc * CHUNK + j * MM
                        nc.tensor.matmul(ps[:, j * MM:(j + 1) * MM], lhsT,
                                         xt[:, c0:c0 + MM], start=True, stop=True)
                    e_tmp = etmp_pool.tile([128, CHUNK], dt.float32, tag="et")
                    nc.scalar.activation(e_tmp[:], ps[:], AF.Exp, bias=0.0,
                                         scale=INV_T,
                                         accum_out=sparts[b][:, kc:kc + 1])
                # band: sims for the W-wide positive window
                psb = psum.tile([128, W], dt.float32, tag="ps")
                for j in range(nwc):
                    nc.tensor.matmul(psb[:, j * MM:(j + 1) * MM], lhsT,
                                     xtband[:, b * W + j * MM: b * W + (j + 1) * MM],
                                     start=True, stop=True)
                nc.scalar.activation(e_band[b][:], psb[:], AF.Exp, bias=0.0,
                                     scale=INV_T)
                nc.vector.tensor_copy(s_band[b][:], psb[:])
                nc.vector.reduce_sum(S[b][:], sparts[b][:], axis=X)

            # ---- Phase B: range masks + positive-window sums (DVE only) ----
            tmp_pool = sp_pool
            for b in range(BLOCKS):
                m1 = tmp_pool.tile([128, W], dt.float32, tag="m1")
                nc.vector.tensor_scalar(m1[:], iota_f[:], gsr[:, b:b + 1], None,
                                        op0=ALU.is_ge)
                nc.vector.scalar_tensor_tensor(mask[b][:], iota_f[:],
                                               ger[:, b:b + 1], m1[:],
                                               op0=ALU.is_lt, op1=ALU.mult)
                ttmp = tmp_pool.tile([128, W], dt.float32, tag="tt")
                nc.vector.tensor_mul(ttmp[:], e_band[b][:], mask[b][:])
                nc.vector.reduce_sum(P[b][:], ttmp[:], axis=X)
                # unsim = S - P  (reuse P tile as unsim)
                nc.vector.tensor_sub(P[b][:], S[b][:], P[b][:])

            # ---- Phase C: u = log(unsim), runsim = 1/unsim ----
            # exp(sim/T - u) == e_band * runsim, so phase D needs no Exp at
            # all: the ACT stream is all-Exp (phase A) then all-Ln, keeping
            # one activation-table set loaded per phase (2 loads total).
            for b in range(BLOCKS):
                nc.scalar.activation(u[b][:], P[b][:], AF.Ln)
            for b in range(BLOCKS):
                nc.vector.reciprocal(runsim[b][:], P[b][:])

            # ---- Phase D: softplus terms via Ln(1 + e*runsim) (Ln table) ----
            E5 = float(np.exp(5.0))
            sp_tiles = []
            for b in range(BLOCKS):
                t2 = small.tile([128, 1], dt.float32, name=f"t2{b}")
                nc.vector.tensor_scalar_mul(t2[:], runsim[b][:], E5)
                nc.scalar.activation(spd[b][:], t2[:], AF.Ln, bias=1.0)
                et = sp_pool.tile([128, W], dt.float32, tag="spe")
                nc.vector.tensor_scalar(et[:], e_band[b][:], runsim[b][:], None,
                                        op0=ALU.mult)
                sp = sp_pool.tile([128, W], dt.float32, tag="spt")
                nc.scalar.activation(sp[:], et[:], AF.Ln, bias=1.0)
                sp_tiles.append(sp)

                # ---- Phase E interleaved (DVE): A, B, combine ----
                A = small.tile([128, 1], dt.float32, name=f"A{b}")
                B = small.tile([128, 1], dt.float32, name=f"B{b}")
                ttmp = tmp_pool.tile([128, W], dt.float32, tag="tt")
                nc.vector.tensor_mul(ttmp[:], sp[:], mask[b][:])
                nc.vector.reduce_sum(A[:], ttmp[:], axis=X)
                ttmp2 = tmp_pool.tile([128, W], dt.float32, tag="tt")
                nc.vector.tensor_mul(ttmp2[:], s_band[b][:], mask[b][:])
                nc.vector.reduce_sum(B[:], ttmp2[:], axis=X)
                # loss = npos*u + A - spd - (INV_T*B - INV_T)
                r1 = small.tile([128, 1], dt.float32, name=f"r1{b}")
                nc.vector.scalar_tensor_tensor(r1[:], u[b][:], npos[:, b:b + 1],
                                               A[:], op0=ALU.mult, op1=ALU.add)
                r2 = small.tile([128, 1], dt.float32, name=f"r2{b}")
                nc.vector.tensor_scalar(r2[:], B[:], INV_T, -INV_T,
                                        op0=ALU.mult, op1=ALU.add)
                r3 = small.tile([128, 1], dt.float32, name=f"r3{b}")
                nc.vector.tensor_add(r3[:], r2[:], spd[b][:])
                nc.vector.tensor_sub(acc[:, b:b + 1], r1[:], r3[:])

            nc.sync.dma_start(out_d[:], acc[:])

    nc.compile()
    return nc


def _prep(input, label):
    """Host-side shard prep: sort by label, normalize, build per-core inputs."""
    import ml_dtypes

    x = np.asarray(input, dtype=np.float32).reshape(N, D)
    lab = np.asarray(label).astype(np.int64).reshape(N)

    order = np.argsort(lab, kind="stable")
    xs, ls = x[order], lab[order]
    counts = np.bincount(ls, minlength=NCLASS)
    n_pos = int((counts.astype(np.int64) ** 2).sum()) - N
    ends = np.cumsum(counts)
    starts = ends - counts
    row_gs = starts[ls]          # [N] group start col per (sorted) row
    row_ge = ends[ls]            # [N] group end col per row

    norms = np.sqrt((xs * xs).sum(1, dtype=np.float32)).astype(np.float32)
    # reference divides by max(n_i*n_j, EPS); for this data the max never
    # binds (norms ~ 11), so plain normalization is exact.
    assert float(norms.min()) ** 2 > EPS * 1.0001
    xn = (xs / norms[:, None]).astype(np.float32)
    xt = np.ascontiguousarray(xn.T).astype(ml_dtypes.bfloat16)  # [128, N]

    # band windows per global block
    nblk = N // 128
    lo = row_gs[np.arange(nblk) * 128]
    hi = row_ge[np.arange(nblk) * 128 + 127]
    maxband = int((hi - lo).max())
    W = max(512, ((maxband + 511) // 512) * 512)
    wstart = np.minimum(lo, N - W)

    in_maps = []
    for c in range(NCORES):
        r0 = c * ROWS_PER_CORE
        xtband = np.empty((128, BLOCKS * W), dtype=ml_dtypes.bfloat16)
        gsr = np.empty((128, BLOCKS), np.float32)
        ger = np.empty((128, BLOCKS), np.float32)
        npos = np.empty((128, BLOCKS), np.float32)
        for b in range(BLOCKS):
            g = c * BLOCKS + b
            ws = int(wstart[g])
            xtband[:, b * W:(b + 1) * W] = xt[:, ws:ws + W]
            rows = slice(r0 + b * 128, r0 + (b + 1) * 128)
            gsr[:, b] = (row_gs[rows] - ws).astype(np.float32)
            ger[:, b] = (row_ge[rows] - ws).astype(np.float32)
            npos[:, b] = (row_ge[rows] - row_gs[rows] - 1).astype(np.float32)
        in_maps.append({
            "xt": xt,
            "xtown": np.ascontiguousarray(
                xt[:, r0:r0 + ROWS_PER_CORE]),
            "xtband": xtband,
            "gsr": gsr,
            "ger": ger,
            "npos": npos,
        })
    return in_maps, n_pos, W


def kernel(input, label):
    from concourse.bass_utils import run_bass_kernel_spmd

    in_maps, n_pos, W = _prep(input, label)
    if W not in _CACHE:
        _CACHE[W] = _build_nc(W)
    nc = _CACHE[W]

    res = None
    for attempt in range(4):
        try:
            res = run_bass_kernel_spmd(nc, in_maps, core_ids=list(range(NCORES)))
            break
        except Exception:
            if attempt == 3:
                raise
            import time
            time.sleep(45)  # device may need a moment to recover
    global LAST_RESULTS
    LAST_RESULTS = res
    total = 0.0
    for r in res.results:
        total += float(np.sum(r["out"], dtype=np.float64))
    return np.array(total / n_pos, dtype=np.float32)


LAST_RESULTS = None



# revision 7
# speedup vs baseline: 1.5883x; 1.5883x over previous
"""Trainium2 Bass kernel for nn_ContrastiveLoss (NT-Xent style contrastive loss).

Strategy (8 NeuronCores, SPMD):
  - Host sorts samples by label (the scalar loss is permutation invariant),
    row-normalizes, and builds X^T [D=128, N=8192] in bf16.
  - Rows are sharded across 8 cores (1024 rows each, 8 blocks of 128).
  - Each core computes its [1024, 8192] similarity block against the full
    X^T, reduces exp-row-sums on-chip via ACT-accum, and evaluates the
    positive-pair terms on a narrow label-band window (sorted labels make
    positives contiguous).
  - The ACT (scalar) engine is the bottleneck, so it runs *only* Exp plus
    two trailing Lns (2 activation-table loads total):
      * all 8 blocks' softplus inputs t = 1 + e*mask/unsim are collected in
        one [128, 8W] buffer; ONE Ln-with-accum sums ln(t) per partition
        (masked-out entries are exactly 1 -> ln contributes 0; the host only
        needs the total loss so cross-block accumulation is fine)
      * the linear term (sum_range sim/T - 1/T) is data-independent of the
        exp sums and is precomputed exactly on the host (hostL).
  - Per-row partial losses return to the host, which sums them and divides
    by the exact positive-pair count (from the label histogram).

Math: with e_ij = exp(sim_ij/T), S_i = sum_j e_ij (incl diag),
P_i = sum_{mask} e_ij (incl diag), unsim_i = S_i - P_i, u_i = log(unsim_i),
r_i = 1/unsim_i, the reference loss row-sum equals
  npos_i*u_i + sum_mask ln(1 + e_ij*r_i) - ln(1 + e^5*r_i) - hostL_i
where hostL_i = (sum_mask sim_ij)/T - 1/T and npos_i = label count - 1.
Diagonal contributions cancel exactly in unsim; the diag terms use the
same bf16 sim_ii in P and S so the subtraction is bitwise consistent.
"""

import numpy as np

T = 0.2
INV_T = 1.0 / T  # 5.0
EPS = 1e-5
N, D, NCLASS = 8192, 128, 128
NCORES = 8
ROWS_PER_CORE = N // NCORES          # 1024
BLOCKS = ROWS_PER_CORE // 128        # 8 blocks of 128 rows per core
CHUNK = 2048                         # ACT chunk (4 PSUM banks)
NCHUNKS = N // CHUNK                 # 4 per block
MM = 512                             # matmul free-dim per PSUM bank

_CACHE = {}


def _build_nc(W, debug=False):
    """Build the SPMD Bass/Tile program. W = band window width (mult of 512)."""
    import concourse.bass as bass
    import concourse.bacc as bacc
    import concourse.mybir as mybir
    import concourse.tile as tile

    dt = mybir.dt
    AF = mybir.ActivationFunctionType
    ALU = mybir.AluOpType
    X = mybir.AxisListType.X

    nc = bacc.Bacc("TRN2", target_bir_lowering=False, debug=debug)

    xt_d = nc.dram_tensor("xt", [128, N], dt.bfloat16, kind="ExternalInput")
    xtown_d = nc.dram_tensor("xtown", [128, ROWS_PER_CORE], dt.bfloat16,
                             kind="ExternalInput")
    xtband_d = nc.dram_tensor("xtband", [128, BLOCKS * W], dt.bfloat16,
                              kind="ExternalInput")
    mask_d = nc.dram_tensor("maskm", [128, BLOCKS * W], dt.bfloat16,
                            kind="ExternalInput")
    npos_d = nc.dram_tensor("npos", [128, BLOCKS], dt.float32, kind="ExternalInput")
    hostl_d = nc.dram_tensor("hostl", [128, BLOCKS], dt.float32,
                             kind="ExternalInput")
    out_d = nc.dram_tensor("out", [128, BLOCKS + 1], dt.float32,
                           kind="ExternalOutput")

    nwc = W // MM  # band matmul sub-chunks
    E5 = float(np.exp(5.0))

    with tile.TileContext(nc) as tc:
        with (
            tc.tile_pool(name="const", bufs=1) as const,
            tc.tile_pool(name="band", bufs=1) as band,
            tc.tile_pool(name="etmp", bufs=3) as etmp_pool,
            tc.tile_pool(name="sp", bufs=2) as sp_pool,
            tc.tile_pool(name="small", bufs=1) as small,
            tc.tile_pool(name="psum", bufs=2, space="PSUM") as psum,
        ):
            # ---- persistent loads (xtown + first xt chunks first) ----
            xtown = const.tile([128, ROWS_PER_CORE], dt.bfloat16)
            nc.sync.dma_start(xtown[:], xtown_d[:])
            xt = const.tile([128, N], dt.bfloat16)
            for k in range(N // 1024):
                nc.sync.dma_start(xt[:, k * 1024:(k + 1) * 1024],
                                  xt_d[:, k * 1024:(k + 1) * 1024])
            xtband = const.tile([128, BLOCKS * W], dt.bfloat16)
            nc.sync.dma_start(xtband[:], xtband_d[:])
            maskm = const.tile([128, BLOCKS * W], dt.bfloat16)
            nc.sync.dma_start(maskm[:], mask_d[:])
            npos = const.tile([128, BLOCKS], dt.float32)
            nc.sync.dma_start(npos[:], npos_d[:])
            hostl = const.tile([128, BLOCKS], dt.float32)
            nc.sync.dma_start(hostl[:], hostl_d[:])

            acc = const.tile([128, BLOCKS + 1], dt.float32)

            # per-block persistent tiles
            e_band = [band.tile([128, W], dt.float32, name=f"eb{b}")
                      for b in range(BLOCKS)]
            tbig = band.tile([128, BLOCKS * W], dt.float32)  # ln inputs, all blocks
            S = [small.tile([128, 1], dt.float32, name=f"S{b}") for b in range(BLOCKS)]
            P = [small.tile([128, 1], dt.float32, name=f"P{b}") for b in range(BLOCKS)]
            r = [small.tile([128, 1], dt.float32, name=f"r{b}") for b in range(BLOCKS)]
            sparts = [small.tile([128, NCHUNKS], dt.float32, name=f"sp{b}")
                      for b in range(BLOCKS)]
            lnin = small.tile([128, 2 * BLOCKS], dt.float32)
            lnout = small.tile([128, 2 * BLOCKS], dt.float32)
            lnbig = band.tile([128, BLOCKS * W], dt.bfloat16)  # scrap Ln output

            # ---- Phase A: per block: dense exp row-sums + band exp ----
            # ACT stream is Exp-only here; the single batched Ln below is the
            # only other table function -> 2 ACT_TABLE_LOADs total.
            for b in range(BLOCKS):
                lhsT = xtown[:, b * 128:(b + 1) * 128]
                for kc in range(NCHUNKS):
                    ps = psum.tile([128, CHUNK], dt.float32, tag="ps")
                    for j in range(CHUNK // MM):
                        c0 = kc * CHUNK + j * MM
                        nc.tensor.matmul(ps[:, j * MM:(j + 1) * MM], lhsT,
                                         xt[:, c0:c0 + MM], start=True, stop=True)
                    e_tmp = etmp_pool.tile([128, CHUNK], dt.bfloat16, tag="et")
                    nc.scalar.activation(e_tmp[:], ps[:], AF.Exp, bias=0.0,
                                         scale=INV_T,
                                         accum_out=sparts[b][:, kc:kc + 1])
                # band: sims for the W-wide positive window
                psb = psum.tile([128, W], dt.float32, tag="ps")
                for j in range(nwc):
                    nc.tensor.matmul(psb[:, j * MM:(j + 1) * MM], lhsT,
                                     xtband[:, b * W + j * MM: b * W + (j + 1) * MM],
                                     start=True, stop=True)
                nc.scalar.activation(e_band[b][:], psb[:], AF.Exp, bias=0.0,
                                     scale=INV_T)

                # ---- DVE per block (overlaps later blocks' ACT work) ----
                nc.vector.reduce_sum(S[b][:], sparts[b][:], axis=X)
                # masked band exp + its sum (one fused pass)
                me = sp_pool.tile([128, W], dt.float32, tag="me")
                nc.vector.scalar_tensor_tensor(
                    me[:], e_band[b][:], 1.0, maskm[:, b * W:(b + 1) * W],
                    op0=ALU.mult, op1=ALU.mult, accum_out=P[b][:])
                # unsim -> lnin col b (u = Ln later)
                nc.vector.tensor_sub(lnin[:, b:b + 1], S[b][:], P[b][:])
                nc.vector.reciprocal(r[b][:], lnin[:, b:b + 1])
                # spd input: 1 + e^5 * r  -> lnin col 8+b
                nc.vector.tensor_scalar(lnin[:, BLOCKS + b:BLOCKS + b + 1],
                                        r[b][:], E5, 1.0,
                                        op0=ALU.mult, op1=ALU.add)
                # t = me*r + 1 ; masked-out entries give exactly 1
                nc.vector.tensor_scalar(tbig[:, b * W:(b + 1) * W], me[:],
                                        r[b][:], 1.0, op0=ALU.mult, op1=ALU.add)

            # ---- Phase B: Lns (the only table switch; both run after all
            # Exps because tbig needs every block's DVE pass) ----
            atot = small.tile([128, 1], dt.float32)
            nc.scalar.activation(lnbig[:], tbig[:], AF.Ln, accum_out=atot[:])
            nc.scalar.activation(lnout[:], lnin[:], AF.Ln)

            # ---- Phase C: batched combine on DVE ----
            # loss = npos*u - (spd + hostL)   (per block cols 0..7)
            # col 8 = per-partition sum of all softplus terms (ln t)
            t1 = small.tile([128, BLOCKS], dt.float32)
            t3 = small.tile([128, BLOCKS], dt.float32)
            nc.vector.tensor_mul(t1[:], npos[:], lnout[:, 0:BLOCKS])
            nc.vector.tensor_add(t3[:], lnout[:, BLOCKS:2 * BLOCKS], hostl[:])
            nc.vector.tensor_sub(acc[:, 0:BLOCKS], t1[:], t3[:])
            nc.vector.tensor_copy(acc[:, BLOCKS:BLOCKS + 1], atot[:])

            nc.sync.dma_start(out_d[:], acc[:])

    nc.compile()
    return nc


def _prep(input, label):
    """Host-side shard prep: sort by label, normalize, build per-core inputs."""
    import ml_dtypes

    x = np.asarray(input, dtype=np.float32).reshape(N, D)
    lab = np.asarray(label).astype(np.int64).reshape(N)

    order = np.argsort(lab, kind="stable")
    xs, ls = x[order], lab[order]
    counts = np.bincount(ls, minlength=NCLASS)
    n_pos = int((counts.astype(np.int64) ** 2).sum()) - N
    ends = np.cumsum(counts)
    starts = ends - counts
    row_gs = starts[ls]          # [N] group start col per (sorted) row
    row_ge = ends[ls]            # [N] group end col per row

    norms = np.sqrt((xs * xs).sum(1, dtype=np.float32)).astype(np.float32)
    # reference divides by max(n_i*n_j, EPS); for this data the max never
    # binds (norms ~ 11), so plain normalization is exact.
    assert float(norms.min()) ** 2 > EPS * 1.0001
    xn = (xs / norms[:, None]).astype(np.float32)
    xb = xn.astype(ml_dtypes.bfloat16)
    xt = np.ascontiguousarray(xb.T)                     # [128, N] bf16
    xf = xb.astype(np.float32)                          # bf16-rounded values

    # hostL: (sum over same-label cols of sim)/T - 1/T, from per-class sums
    z = np.zeros((NCLASS, D), np.float64)
    np.add.at(z, ls, xf.astype(np.float64))
    B = np.einsum('nd,nd->n', xf.astype(np.float64), z[ls])
    hostL_rows = (INV_T * B - INV_T).astype(np.float32)  # [N]

    # band windows per global block
    nblk = N // 128
    lo = row_gs[np.arange(nblk) * 128]
    hi = row_ge[np.arange(nblk) * 128 + 127]
    maxband = int((hi - lo).max())
    W = max(512, ((maxband + 511) // 512) * 512)
    wstart = np.minimum(lo, N - W)

    iot = np.arange(W, dtype=np.int64)[None, :]

    in_maps = []
    for c in range(NCORES):
        r0 = c * ROWS_PER_CORE
        xtband = np.empty((128, BLOCKS * W), dtype=ml_dtypes.bfloat16)
        maskm = np.empty((128, BLOCKS * W), dtype=ml_dtypes.bfloat16)
        npos = np.empty((128, BLOCKS), np.float32)
        hostl = np.empty((128, BLOCKS), np.float32)
        for b in range(BLOCKS):
            g = c * BLOCKS + b
            ws = int(wstart[g])
            xtband[:, b * W:(b + 1) * W] = xt[:, ws:ws + W]
            rows = slice(r0 + b * 128, r0 + (b + 1) * 128)
            gs = (row_gs[rows] - ws)[:, None]
            ge = (row_ge[rows] - ws)[:, None]
            maskm[:, b * W:(b + 1) * W] = ((iot >= gs) & (iot < ge)).astype(
                ml_dtypes.bfloat16)
            npos[:, b] = (row_ge[rows] - row_gs[rows] - 1).astype(np.float32)
            hostl[:, b] = hostL_rows[rows]
        in_maps.append({
            "xt": xt,
            "xtown": np.ascontiguousarray(
                xt[:, r0:r0 + ROWS_PER_CORE]),
            "xtband": xtband,
            "maskm": maskm,
            "npos": npos,
            "hostl": hostl,
        })
    return in_maps, n_pos, W


def kernel(input, label):
    from concourse.bass_utils import run_bass_kernel_spmd

    in_maps, n_pos, W = _prep(input, label)
    if W not in _CACHE:
        _CACHE[W] = _build_nc(W)
    nc = _CACHE[W]

    res = None
    for attempt in range(4):
        try:
            res = run_bass_kernel_spmd(nc, in_maps, core_ids=list(range(NCORES)))
            break
        except Exception:
            if attempt == 3:
                raise
            import time
            time.sleep(45)  # device may need a moment to recover
    global LAST_RESULTS
    LAST_RESULTS = res
    total = 0.0
    for r in res.results:
        total += float(np.sum(r["out"], dtype=np.float64))
    return np.array(total / n_pos, dtype=np.float32)


LAST_RESULTS = None
